# revision 58
# baseline (speedup 1.0000x reference)
"""Deformable-conv-2d (adaptive dilation) Trainium2 Bass kernel.

Full-input contract: kernel(**inputs) takes the unsharded reference inputs
and returns the full (4, 256, 64, 64) float32 output.

Sharding: data-parallel over (batch sample x H-half) across 8 cores.
Core k handles sample b = k // 2, output rows [32*(k%2), 32*(k%2)+32).

Per-core device pipeline (v2: per-half software pipelining):
  per half (16 output rows = 1024 positions):
  1. small convs (offset 18ch / mask 9ch / adaptive-dilation 3ch) as 9
     shifted matmuls (fp32r) accumulated in PSUM.
  2. PE-transpose conv outputs to position-on-partition layout, then an
     elementwise fp32 pipeline (floor via mod, snap masks, clamps) produces
     per-sample-point gather indices + 4 bilinear corner coefficients.
  3. index relayout to the SWDGE wrapped format via a PE partition-fold
     matmul (identity-column selectors) + one DVE permute/cast copy +
     7 parallel replication DMAs across both HWDGE rings.
  4. per tap, two 512-idx dma_gathers (2KB rows = 2x2 corner patch x 256
     ch, bf16) from an HBM table prepared on host; fine gather granularity
     keeps gb-buffer WAR waits from backing up the SWDGE ring.
  5. combine corners with scalar_tensor_tensor chains (coef = per-partition
     scalar), PE-transpose back to channel-major; PSUM->SBUF copies are
     deferred one tap so they stay off the ACT critical path.
  6. big conv = bf16 matmuls (K = 256c x 9 taps) accumulating in PSUM.
"""

import numpy as np
import ml_dtypes

import concourse.bacc as bacc
import concourse.mybir as mybir
import concourse.tile as tile
from concourse import library_config
from concourse.bass_utils import run_bass_kernel_spmd

DT = mybir.dt
ALU = mybir.AluOpType
ACTF = mybir.ActivationFunctionType

B, C, OC, H, W = 4, 256, 256, 64, 64
KS, PAD, DIL = 3, 1, 2
N = KS * KS                       # 9 kernel points
HP = H + 2 * PAD                  # 66 (reference x_pad height/width)
NCORES = 8
ROWS = H // 2                     # 32 output rows per core
S = ROWS * W                      # 2048 output positions per core
CHUNKS = S // 128                 # 16 s-chunks of 128
HCH = CHUNKS // 2                 # 8 chunks per half
GROWS = HP * HP                   # 4356 gather-table rows
GE = 4 * C                        # 1024 bf16 elems per gather row

_CACHED = {}


def _build_program():
    nc = bacc.Bacc("TRN2", target_bir_lowering=False, debug=False,
                   num_devices=NCORES, num_swdge_queues=1)

    slab = nc.dram_tensor("slab", [2, 128, 34, HP], DT.float32r, kind="ExternalInput")
    rrows = nc.dram_tensor("rrows", [GROWS, GE], DT.bfloat16, kind="ExternalInput")
    wsm = nc.dram_tensor("wsm", [128, N, 2, 30], DT.float32r, kind="ExternalInput")
    biasd = nc.dram_tensor("biasd", [30, 1], DT.float32, kind="ExternalInput")
    p0xd = nc.dram_tensor("p0xd", [128, CHUNKS, N], DT.float32, kind="ExternalInput")
    p0yd = nc.dram_tensor("p0yd", [128, CHUNKS, N], DT.float32, kind="ExternalInput")
    wcv = nc.dram_tensor("wcv", [128, N, 2, OC], DT.bfloat16, kind="ExternalInput")
    eyebd = nc.dram_tensor("eyebd", [128, 128], DT.bfloat16, kind="ExternalInput")
    eyefd = nc.dram_tensor("eyefd", [128, 128], DT.float32, kind="ExternalInput")
    outd = nc.dram_tensor("out", [2, 128, S], DT.float32, kind="ExternalOutput")

    with tile.TileContext(nc) as tc:
        with (
            tc.tile_pool(name="const", bufs=1) as cpool,
            tc.tile_pool(name="work", bufs=1) as wpool,
            tc.tile_pool(name="pipe", bufs=1) as ppool,
            tc.tile_pool(name="gath", bufs=12) as gpool,
            tc.tile_pool(name="xo", bufs=2) as xopool,
            tc.tile_pool(name="xot", bufs=4) as xotpool,
            tc.tile_pool(name="osb", bufs=2) as opool,
            tc.tile_pool(name="acc", bufs=1, space="PSUM") as acc_pool,
            tc.tile_pool(name="ps2", bufs=2, space="PSUM") as ps2_pool,
        ):
            # preload the gather ucode library up front so the first real
            # dma_gather doesn't pay the UNLOAD/LOAD + ~6us IRAM penalty on
            # the critical path (the gpsimd queue carries only gathers)
            nc.gpsimd.load_library(library_config.mlp)

            # ---- static loads -------------------------------------------------
            slab_sb = []
            for g in range(2):
                sl = cpool.tile([128, 34, HP], DT.float32r, tag=f"slab{g}",
                                name=f"slab_sb{g}")
                nc.sync.dma_start(sl[:], slab[g])
                slab_sb.append(sl)

            wsm_sb = cpool.tile([128, N, 2, 30], DT.float32r, tag="wsm")
            nc.sync.dma_start(wsm_sb[:], wsm.ap())
            eyef = cpool.tile([128, 128], DT.float32, tag="eyef")
            nc.sync.dma_start(eyef[:], eyefd.ap())

            bias_sb = cpool.tile([30, 1], DT.float32, tag="bias")
            nc.scalar.dma_start(bias_sb[:], biasd.ap())
            p0x_sb = cpool.tile([128, CHUNKS, N], DT.float32, tag="p0x")
            nc.scalar.dma_start(p0x_sb[:], p0xd.ap())
            p0y_sb = cpool.tile([128, CHUNKS, N], DT.float32, tag="p0y")
            nc.scalar.dma_start(p0y_sb[:], p0yd.ap())
            eyeb = cpool.tile([128, 128], DT.bfloat16, tag="eyeb")
            nc.scalar.dma_start(eyeb[:], eyebd.ap())
            # wcv is only needed by the first conv matmul (~70us in) — load last
            wcv_sb = cpool.tile([128, N, 2, OC], DT.bfloat16, tag="wcv")
            nc.scalar.dma_start(wcv_sb[:], wcv.ap())

            # const tiles for STT-form pipeline ops (immune to the DVE
            # 2-port-mode / GpSimd SBUF port conflict that stalls
            # tensor_scalar during gather descriptor generation)
            c0t = cpool.tile([128, HCH, N], DT.float32, tag="c0t")
            nc.vector.memset(c0t[:], 0.0)
            c1t = cpool.tile([128, HCH, N], DT.float32, tag="c1t")
            nc.vector.memset(c1t[:], 1.0)
            c65t = cpool.tile([128, HCH, N], DT.float32, tag="c65t")
            nc.vector.memset(c65t[:], float(HP - 1))
            c76t = cpool.tile([128, HCH, N], DT.float32, tag="c76t")
            nc.vector.memset(c76t[:], 76.0)
            cm4t = cpool.tile([128, HCH, 3], DT.float32, tag="cm4t")
            nc.vector.memset(cm4t[:], -4.0)

            # PE warm-up burst: ~5us of tiny matmuls on const data while the
            # input DMAs land, so the HAM throttle is at 2.4GHz before the
            # small conv starts (cold smallconv costs ~8us extra)
            warm = ps2_pool.tile([8, 16], DT.float32, tag="ps2", name="warm")
            for _ in range(25):
                nc.tensor.matmul(warm[:], c0t[:, 0, 0:8], c0t[:, 0:2, 0:8],
                                 start=True, stop=True)

            # ================== per-half index/coef preparation ===============
            def half_prep(sh):
                # small conv: PSUM [30, 1024], 9 taps x 2 cgroups x 2 blocks
                psc = ps2_pool.tile([30, 1024], DT.float32, tag="ps2",
                                    name=f"psc{sh}")
                for b in range(2):
                    blk = sh * 2 + b
                    first = True
                    # g outer: the first 9 matmuls need only slab[0], so the
                    # PE can start before slab[1]'s DMA lands
                    for g in range(2):
                        for t in range(N):
                            ki, kj = t // 3, t % 3
                            rhs = slab_sb[g][:, blk * 8 + ki: blk * 8 + ki + 8,
                                             kj: kj + W]
                            nc.tensor.matmul(
                                psc[:, b * 512:(b + 1) * 512],
                                wsm_sb[:, t, g, :],
                                rhs,
                                start=first, stop=(t == N - 1 and g == 1))
                            first = False

                conv_sb = wpool.tile([30, 1024], DT.float32, tag=f"conv{sh}")
                nc.scalar.activation(conv_sb[:], psc[:], ACTF.Identity,
                                     bias=bias_sb[:])

                # transpose -> convT [128, HCH, 30]
                pst = ps2_pool.tile([128, HCH * 30], DT.float32, tag="ps2",
                                    name=f"pst{sh}")
                for ch in range(HCH):
                    nc.tensor.transpose(pst[:, ch * 30:(ch + 1) * 30],
                                        conv_sb[:, ch * 128:(ch + 1) * 128],
                                        eyef[:30, :30])
                convT = wpool.tile([128, HCH, 30], DT.float32, tag=f"convT{sh}")
                nc.scalar.copy(convT.rearrange("p a b -> p (a b)"), pst[:])

                # ---- index & coefficient pipeline (fp32) ---------------------
                def t9(tag):
                    return ppool.tile([128, HCH, N], DT.float32,
                                      tag=tag + str(sh), name=tag + str(sh))

                def t3_(tag):
                    return ppool.tile([128, HCH, 3], DT.float32,
                                      tag=tag + str(sh), name=tag + str(sh))

                m9 = t9("m9")
                nc.scalar.activation(m9[:], convT[:, :, 18:27], ACTF.Sigmoid)
                adb = t3_("adb")
                nc.scalar.activation(adb[:], convT[:, :, 27:30], ACTF.Sigmoid,
                                     scale=-1.0)
                t3 = t3_("t3")
                nc.vector.scalar_tensor_tensor(t3[:], adb[:], 2.0,
                                               c1t[:, :, 0:3], ALU.mult, ALU.add)
                adm = t3_("adm")
                nc.vector.scalar_tensor_tensor(adm[:], t3[:], 2.0,
                                               cm4t[:], ALU.mult, ALU.add)
                mu = t9("mu")
                for k in range(3):
                    nc.vector.tensor_tensor(mu[:, :, 3 * k:3 * k + 3],
                                            m9[:, :, 3 * k:3 * k + 3],
                                            adm[:, :, 0:3], ALU.mult)

                def axis_idx(off_lo, p0_sb, is_x, tagp):
                    # index-critical part only: p -> floor -> ql (feeds idxf)
                    p = t9(tagp + "p")
                    nc.vector.tensor_tensor(p[:], convT[:, :, off_lo:off_lo + N],
                                            p0_sb[:, sh * HCH:(sh + 1) * HCH, :],
                                            ALU.add)
                    if is_x:
                        nc.vector.tensor_tensor(p[:, :, 0:3], p[:, :, 0:3],
                                                t3[:, :, 0:3], ALU.subtract)
                        nc.vector.tensor_tensor(p[:, :, 6:9], p[:, :, 6:9],
                                                t3[:, :, 0:3], ALU.add)
                    else:
                        for n in (0, 3, 6):
                            nc.vector.tensor_tensor(p[:, :, n:n + 1],
                                                    p[:, :, n:n + 1],
                                                    t3[:, :, 0:1], ALU.subtract)
                        for n in (2, 5, 8):
                            nc.vector.tensor_tensor(p[:, :, n:n + 1],
                                                    p[:, :, n:n + 1],
                                                    t3[:, :, 2:3], ALU.add)
                    nc.vector.scalar_tensor_tensor(p[:], p[:], -10.0, c76t[:],
                                                   ALU.max, ALU.min)
                    tmp = t9(tagp + "tmp")
                    # floor(p): t = int-round(p) (any within-1 rounding), then
                    # f = t - (t > p)
                    ti = ppool.tile([128, HCH, N], DT.int32,
                                    tag=tagp + "ti" + str(sh),
                                    name=tagp + "ti" + str(sh))
                    nc.scalar.copy(ti[:], p[:])
                    f = t9(tagp + "f")
                    nc.scalar.copy(f[:], ti[:])
                    nc.vector.tensor_tensor(tmp[:], f[:], p[:], ALU.is_gt)
                    nc.vector.tensor_tensor(f[:], f[:], tmp[:], ALU.subtract)
                    ql = t9(tagp + "ql")
                    nc.vector.scalar_tensor_tensor(ql[:], f[:], 0.0, c65t[:],
                                                   ALU.max, ALU.min)
                    return p, f, ql, tmp

                def axis_coef(st, tagp):
                    # bilinear coefficient tail, deferred past the index fold
                    p, f, ql, tmp = st
                    ma = t9(tagp + "ma")
                    nc.vector.scalar_tensor_tensor(ma[:], p[:], 1.0, c0t[:],
                                                   ALU.is_lt, ALU.add)
                    nc.vector.scalar_tensor_tensor(tmp[:], p[:], float(HP - 2),
                                                   c0t[:], ALU.is_gt, ALU.add)
                    nc.vector.tensor_tensor(ma[:], ma[:], tmp[:], ALU.add)
                    # snapped & clipped p_used: pu = p + ma*(ql - p)
                    # (equals clip(where(mask, floor, p), 0, 65): unmasked p is
                    # already in [1, 64], masked case is exactly ql)
                    nc.vector.tensor_tensor(tmp[:], ql[:], p[:], ALU.subtract)
                    nc.vector.tensor_tensor(tmp[:], ma[:], tmp[:], ALU.mult)
                    pu = t9(tagp + "pu")
                    nc.vector.tensor_tensor(pu[:], p[:], tmp[:], ALU.add)
                    qr = t9(tagp + "qr")
                    nc.vector.scalar_tensor_tensor(qr[:], f[:], 1.0, c0t[:],
                                                   ALU.add, ALU.max)
                    nc.vector.scalar_tensor_tensor(qr[:], qr[:], 0.0, c65t[:],
                                                   ALU.add, ALU.min)
                    ex = t9(tagp + "ex")
                    nc.vector.tensor_tensor(ex[:], qr[:], ql[:], ALU.is_equal)
                    wl = t9(tagp + "wl")
                    nc.vector.scalar_tensor_tensor(wl[:], ql[:], 1.0, pu[:],
                                                   ALU.add, ALU.subtract)
                    wrt = t9(tagp + "wrt")
                    nc.vector.scalar_tensor_tensor(wrt[:], pu[:], 1.0, qr[:],
                                                   ALU.add, ALU.subtract)
                    nc.vector.tensor_tensor(tmp[:], wrt[:], ex[:], ALU.mult)
                    c0 = t9(tagp + "c0")
                    nc.vector.tensor_tensor(c0[:], wl[:], tmp[:], ALU.add)
                    c1 = t9(tagp + "c1")
                    nc.vector.tensor_tensor(c1[:], wrt[:], tmp[:], ALU.subtract)
                    return c0, c1

                stx = axis_idx(0, p0x_sb, True, "x")
                sty = axis_idx(N, p0y_sb, False, "y")
                qlx, qly = stx[2], sty[2]

                idxf = t9("idxf")
                nc.vector.scalar_tensor_tensor(idxf[:], qlx[:], float(HP),
                                               qly[:], ALU.mult, ALU.add)

                # ---- wrapped-index relayout via PE partition fold ------------
                # wrh[q, n, ch, pg] = idxf[16*pg + q, ch, n]
                wrh = wpool.tile([128, N, HCH, 8], DT.int16, tag=f"wr{sh}")
                idxr = idxf.rearrange("p a b -> p (a b)")
                for grp in range(2):
                    fps = ps2_pool.tile([16, 4, HCH * N], DT.float32, tag="ps2",
                                        name=f"fold{sh}{grp}")
                    for j in range(4):
                        pp = grp * 4 + j
                        nc.tensor.matmul(
                            fps[:, j, :],
                            eyef[:, 16 * pp:16 * pp + 16],
                            idxr,
                            start=True, stop=True)
                    # permute (pg', ch, n) -> (n, ch, pg') with int16 cast
                    nc.scalar.copy(
                        wrh[0:16, :, :, grp * 4:(grp + 1) * 4],
                        fps.rearrange("q g (c n) -> q n c g", c=HCH, n=N))
                # replicate across the 8 gpsimd cores (7 parallel copies on
                # the two HWDGE rings)
                for k in range(1, 8):
                    eng = nc.sync if k % 2 else nc.scalar
                    eng.dma_start(wrh[16 * k:16 * k + 16], wrh[0:16])

                # bilinear corner coefficients (needed only by the combine,
                # so computed after the index path unblocks the first gather)
                cx0, cx1 = axis_coef(stx, "x")
                cy0, cy1 = axis_coef(sty, "y")
                w0 = t9("w0")
                nc.vector.tensor_tensor(w0[:], mu[:], cx0[:], ALU.mult)
                w1 = t9("w1")
                nc.vector.tensor_tensor(w1[:], mu[:], cx1[:], ALU.mult)
                u00 = t9("u00")
                nc.vector.tensor_tensor(u00[:], w0[:], cy0[:], ALU.mult)
                u01 = t9("u01")
                nc.vector.tensor_tensor(u01[:], w0[:], cy1[:], ALU.mult)
                u10 = t9("u10")
                nc.vector.tensor_tensor(u10[:], w1[:], cy0[:], ALU.mult)
                u11 = t9("u11")
                nc.vector.tensor_tensor(u11[:], w1[:], cy1[:], ALU.mult)

                return wrh, u00, u01, u10, u11

            # ================== per-half gather + conv main loop ==============
            pso = acc_pool.tile([128, 2 * 1024], DT.float32, tag="acc",
                                name="pso")

            def half_exec(sh, prep, gi0):
                wrh, u00, u01, u10, u11 = prep

                def flush(pend):
                    # deferred PSUM->SBUF copies + conv matmuls for tap n-1:
                    # emitted after tap n's ACT muls so the copies don't block
                    # the combine's critical path on the ACT queue
                    pn, pts = pend
                    for g in range(2):
                        xoT = xotpool.tile([128, 1024], DT.bfloat16,
                                           tag="xoT", name="xoT")
                        nc.scalar.copy(xoT[:], pts[g][:])
                        for og in range(2):
                            for nb in range(2):
                                nc.tensor.matmul(
                                    pso[:, og * 1024 + nb * 512:
                                        og * 1024 + (nb + 1) * 512],
                                    wcv_sb[:, pn, g, og * 128:(og + 1) * 128],
                                    xoT[:, nb * 512:(nb + 1) * 512],
                                    start=(pn == 0 and g == 0),
                                    stop=(pn == N - 1 and g == 1))

                pending = None
                for n in range(N):
                    # two 512-idx sub-gathers per tap: finer WAR granularity
                    # keeps the gb-buffer wait from stalling the SWDGE ring
                    # mid-instruction
                    xos = []
                    for hb in range(2):
                        gb = gpool.tile([128, HCH // 2, GE], DT.bfloat16,
                                        tag="gb", name="gb")
                        nc.gpsimd.dma_gather(
                            gb[:], rrows.ap(),
                            wrh[:, n, hb * 4:(hb + 1) * 4],
                            num_idxs=512, num_idxs_reg=512, elem_size=GE,
                            queue_num=0)
                        # corner combine, balanced across ACT and DVE:
                        # 3 of 4 chunks: ACT scales corners 0+2, DVE runs two
                        # independent fused STTs + one bf16 TT add (2x mode);
                        # 1 of 4 chunks: classic 1 ACT mul + 3-STT chain.
                        for c4 in range(HCH // 2):
                            ch = hb * 4 + c4
                            if c4 == 0:
                                xo = xopool.tile([128, C], DT.bfloat16,
                                                 tag="xo", name="xo", bufs=10)
                                nc.scalar.mul(xo[:], gb[:, c4, 0:C],
                                              u00[:, ch, n:n + 1])
                                nc.vector.scalar_tensor_tensor(
                                    xo[:], gb[:, c4, C:2 * C],
                                    u01[:, ch, n:n + 1],
                                    xo[:], ALU.mult, ALU.add)
                                nc.vector.scalar_tensor_tensor(
                                    xo[:], gb[:, c4, 2 * C:3 * C],
                                    u10[:, ch, n:n + 1],
                                    xo[:], ALU.mult, ALU.add)
                                nc.vector.scalar_tensor_tensor(
                                    xo[:], gb[:, c4, 3 * C:4 * C],
                                    u11[:, ch, n:n + 1],
                                    xo[:], ALU.mult, ALU.add)
                                xos.append((xo, None))
                                continue
                            xa = xopool.tile([128, C], DT.bfloat16, tag="xa",
                                             name="xa", bufs=6)
                            nc.scalar.mul(xa[:], gb[:, c4, 0:C],
                                          u00[:, ch, n:n + 1])
                            xb = xopool.tile([128, C], DT.bfloat16, tag="xb",
                                             name="xb", bufs=6)
                            nc.scalar.mul(xb[:], gb[:, c4, 2 * C:3 * C],
                                          u10[:, ch, n:n + 1])
                            nc.vector.scalar_tensor_tensor(
                                xa[:], gb[:, c4, C:2 * C], u01[:, ch, n:n + 1],
                                xa[:], ALU.mult, ALU.add)
                            nc.vector.scalar_tensor_tensor(
                                xb[:], gb[:, c4, 3 * C:4 * C],
                                u11[:, ch, n:n + 1],
                                xb[:], ALU.mult, ALU.add)
                            xo = xopool.tile([128, C], DT.bfloat16, tag="xo",
                                             name="xo", bufs=10)
                            nc.vector.tensor_tensor(xo[:], xa[:], xb[:],
                                                    ALU.add)
                            xos.append((xo, None))
                    if pending is not None:
                        flush(pending)
                    pts = []
                    for g in range(2):
                        pt = ps2_pool.tile([128, 1024], DT.bfloat16, tag="ps2",
                                           name="pt")
                        for ch in range(HCH):
                            t, j = xos[ch]
                            src = (t[:, g * 128:(g + 1) * 128] if j is None
                                   else t[:, j, g * 128:(g + 1) * 128])
                            nc.tensor.transpose(pt[:, ch * 128:(ch + 1) * 128],
                                                src, eyeb[:])
                        pts.append(pt)
                    pending = (n, pts)
                flush(pending)
                for og in range(2):
                    ob = opool.tile([128, 1024], DT.float32, tag="ob", name="ob")
                    nc.scalar.copy(ob[:], pso[:, og * 1024:(og + 1) * 1024])
                    nc.sync.dma_start(outd[og, :, sh * 1024:(sh + 1) * 1024],
                                      ob[:])

            prep0 = half_prep(0)
            prep1 = half_prep(1)
            half_exec(0, prep0, 0)
            half_exec(1, prep1, N)

    nc.compile()
    return nc


def _host_prep(x, w_conv, w_p, b_p, w_m, b_m, w_ad, b_ad):
    bf16 = ml_dtypes.bfloat16
    x = np.asarray(x, dtype=np.float32)
    wsm_full = np.concatenate([np.asarray(w_p), np.asarray(w_m),
                               np.asarray(w_ad)], axis=0).astype(np.float32)
    # [128 part, N, 2, 30] so the device DMA is fully contiguous
    wsm_in = np.ascontiguousarray(
        wsm_full.transpose(2, 3, 1, 0).reshape(N, 2, 128, 30)
        .transpose(2, 0, 1, 3))
    bias_in = np.concatenate([np.asarray(b_p), np.asarray(b_m),
                              np.asarray(b_ad)]).astype(np.float32).reshape(30, 1)
    wcv_in = np.ascontiguousarray(
        np.asarray(w_conv).astype(np.float32).transpose(2, 3, 1, 0)
        .reshape(N, 2, 128, OC).transpose(2, 0, 1, 3)).astype(bf16)
    eyeb = np.eye(128, dtype=np.float32).astype(bf16)
    eyef = np.eye(128, dtype=np.float32)

    in_maps = []
    for k in range(NCORES):
        b, half = k // 2, k % 2
        i0 = ROWS * half
        xp = np.pad(x[b], ((0, 0), (1, 1), (1, 1)))
        slab = np.ascontiguousarray(xp[:, i0:i0 + 34, :]).reshape(2, 128, 34, HP)
        a = np.pad(x[b], ((0, 0), (1, 2), (1, 2))).astype(bf16)
        t = a.transpose(1, 2, 0)                       # (67, 67, 256)
        r4 = np.empty((HP, HP, 4, C), dtype=bf16)
        r4[:, :, 0] = t[0:HP, 0:HP]
        r4[:, :, 1] = t[0:HP, 1:HP + 1]
        r4[:, :, 2] = t[1:HP + 1, 0:HP]
        r4[:, :, 3] = t[1:HP + 1, 1:HP + 1]
        rr = r4.reshape(GROWS, GE)
        sidx = np.arange(S)
        p0x = (1.0 + i0 + sidx // W).astype(np.float32)
        p0y = (1.0 + sidx % W).astype(np.float32)
        # layout [partition, chunk, n]: s = chunk*128 + p
        p0x_t = np.ascontiguousarray(
            np.broadcast_to(p0x.reshape(CHUNKS, 128).T[:, :, None],
                            (128, CHUNKS, N)))
        p0y_t = np.ascontiguousarray(
            np.broadcast_to(p0y.reshape(CHUNKS, 128).T[:, :, None],
                            (128, CHUNKS, N)))
        in_maps.append({
            "slab": slab.astype(np.float32),
            "rrows": rr,
            "wsm": wsm_in,
            "biasd": bias_in,
            "p0xd": p0x_t,
            "p0yd": p0y_t,
            "wcv": wcv_in,
            "eyebd": eyeb,
            "eyefd": eyef,
        })
    return in_maps


def _assemble(results):
    out = np.empty((B, OC, H, W), dtype=np.float32)
    for k in range(NCORES):
        b, half = k // 2, k % 2
        i0 = ROWS * half
        o = np.asarray(results[k]["out"], dtype=np.float32)   # (2, 128, S)
        out[b, :, i0:i0 + ROWS, :] = o.reshape(OC, ROWS, W)
    return out


def run_kernel(inputs, trace=False, **trace_kwargs):
    if "nc" not in _CACHED:
        _CACHED["nc"] = _build_program()
    nc = _CACHED["nc"]
    in_maps = _host_prep(**inputs)
    res = run_bass_kernel_spmd(nc, in_maps, list(range(NCORES)), trace=trace,
                               **trace_kwargs)
    return _assemble(res.results), res


def kernel(**inputs) -> np.ndarray:
    out, _ = run_kernel(inputs)
    return out
